# revision 32
# baseline (speedup 1.0000x reference)
"""Fused AttnBlock kernel for 8 Trainium2 NeuronCores — v2.

Problem: q = LN_head(x1 @ wq), k = LN_head(x2 @ wk), v = x2 @ wv,
out = softmax(q k^T / sqrt(D)) v, with B=4, N=2048, C=1024, H=16, D=64.

Sharding: data-parallel over batch (4) x tensor-parallel over head groups (2).
Each core handles one (batch, head-group) pair fully locally; no collectives.

Key structure vs the v1 baseline (443.7us):
  - k-side LayerNorm algebra: LN'd q rows sum to zero over d, so
    qn . (k - mu_k) == qn . k — the k-centering is dropped entirely, and
    rstd_k is a pure per-token scale folded into the k staging copy.
  - LN stats: one grouped bn_stats per [128,512] tile (8 heads at once,
    emitted with an unoptimized AP so the head grouping survives), even/odd
    halves merged manually; rstd via a DVE bit-trick rsqrt (shift/xor seed +
    one Newton step, ~0.2% max err) — the ACT engine does NOTHING but the
    33.5M softmax exps (256 * [128,4,512] instructions).
  - q/k [n,j] -> [j,n] transposes via the DMA XBAR (16x128-tile transpose
    writing SBUF) — no PE transpose matmuls, no PSUM bounce, no drain copies.
  - attention is block-major: for each 512-wide q block, all 8 heads stream
    before moving on. Block 0 only needs q-proj tiles 0..3, so the exp
    stream starts ~15us in while the rest of q/v projection rides in the
    PE/DVE gaps under it.
  - scores land in a [128,8,512] PSUM ring (4 banks, two 4-slot halves);
    exp reads a whole half [128,4,512] per instruction (2048-elem ACT ops).
  - PV matmuls accumulate [65,512] per block with a ones-column in vA
    producing denominators for free; normalize = approx-reciprocal of the
    denominator row + DRAM-bounce partition broadcast + one multiply.
"""

import os
import sys

for _p in ("/opt/trn_rl_repo",):
    if _p not in sys.path:
        sys.path.insert(0, _p)

import ml_dtypes
import numpy as np

import concourse.bass as bass
import concourse.mybir as mybir
import concourse.tile as tile
from concourse.bass_utils import run_bass_kernel_spmd

F32 = mybir.dt.float32
I32 = mybir.dt.int32
I16 = mybir.dt.int16
BF16 = mybir.dt.bfloat16

B = 4
NSEQ = 2048
DIM = 1024
NHEADS = 16
HDIM = 64
EPS = 1e-5

NCORES = 8
LAST_RESULTS = None
HG = 8            # heads per core
JW = HG * HDIM    # 512 output channels per core
KT = DIM // 128   # 8 contraction tiles for the projections
ALU = mybir.AluOpType


def split_multi_waits(nc, maxw=1):
    # TRN2 instructions carry a single sem-wait slot; this walrus build
    # rejects more. Hoist the excess onto injected NoOps.
    for bb in nc.main_func.blocks:
        new_insts = []
        for inst in bb.instructions:
            si = inst.sync_info
            if si is not None and si.on_wait and len(si.on_wait) > maxw:
                waits = list(si.on_wait)
                extra, keep = waits[:-maxw], waits[-maxw:]
                for ci in range(0, len(extra), maxw):
                    nop = mybir.InstNoOp(
                        name=nc.get_next_instruction_name(), ins=[], outs=[],
                        sync_info=mybir.SyncInfo(
                            on_wait=extra[ci:ci + maxw], on_update=[]),
                    )
                    nop.engine = inst.engine
                    new_insts.append(nop)
                    nc.register_instruction(nop, overwrite=True)
                inst.sync_info = mybir.SyncInfo(
                    on_wait=keep, on_update=list(si.on_update))
            new_insts.append(inst)
        bb.instructions[:] = new_insts


def bn_stats_grouped(nc, out, in_):
    """bn_stats with the head grouping preserved (opt=False APs).

    in_ [128, H, D] -> out [128, H, 6]: per-head (count, mean, n*var) for
    even and odd element halves.
    """
    ve = nc.vector
    inst = mybir.InstBNStats(
        name=nc.get_next_instruction_name(),
        ins=[ve.lower_ap(in_, opt=False)],
        outs=[ve.lower_ap(out, opt=False)],
    )
    return ve.add_instruction(inst)


LOG2E = 1.4426950408889634
EXP_S1 = 128.0 * LOG2E            # Schraudolph bf16 exp: scale
EXP_S2 = 128.0 * (127.0 - 0.0430)  # Schraudolph bf16 exp: bias
# DVE-exp share per block b: how many of the 16 exp instrs per (hp, b)
# ride the DVE (Schraudolph bit-trick) instead of ACT. The DVE is LN-bound
# while projections still stream, so its share ramps up over blocks.
DVE_EXP_SHARE = {0: 0, 1: 4, 2: 5, 3: 5}


def build_fast(n_seq=NSEQ):
    """v3: head-pair attention with row-tiled score matmuls.

    Heads 2t and 2t+1 live on partition halves 0:64 / 64:128 of qT/kT
    column group t. Their score matmuls carry tile_position (0,0) and
    (64,0) (auto-derived from base_partition) so the PE runs both
    concurrently in separate row-groups -> ~2x score throughput vs v2.
    Scores land in a 4-slot PSUM ring as adjacent (even,odd) head pairs;
    one exp instruction covers both heads' [128,2,512] pair. Exp
    alternates between ACT (true exp) and DVE (one tensor_scalar
    f32->int16 round + bf16 bitcast = Schraudolph 2^y, |rel err| <= 3%)
    per DVE_EXP_SHARE, so the serial-ACT exp wall (294us) splits across
    two engines. PV stays M=65 (ones column => free denominators).
    Normalize drain copies moved to ACT; host does the final divide.
    """
    assert n_seq % 512 == 0
    nt_n = n_seq // 128          # 128-token projection tiles
    nblk = n_seq // 512          # 512-wide attention blocks
    assert nt_n % 4 == 0

    nc = bass.Bass()
    x1t = nc.dram_tensor("x1t", [DIM, n_seq], BF16, kind="ExternalInput")
    x2t = nc.dram_tensor("x2t", [DIM, n_seq], BF16, kind="ExternalInput")
    wq_d = nc.dram_tensor("wq", [DIM, JW], BF16, kind="ExternalInput")
    wk_d = nc.dram_tensor("wk", [DIM, JW], BF16, kind="ExternalInput")
    wv_d = nc.dram_tensor("wv", [DIM, JW], BF16, kind="ExternalInput")
    out_d = nc.dram_tensor("out2", [HG, HDIM + 1, n_seq], F32,
                           kind="ExternalOutput")

    x1r = x1t.rearrange("(kt p) n -> p kt n", p=128)
    x2r = x2t.rearrange("(kt p) n -> p kt n", p=128)

    with tile.TileContext(nc) as tc:
        with tc.tile_pool(name="persist", bufs=1) as persist, \
             tc.tile_pool(name="stage", bufs=2) as stage, \
             tc.tile_pool(name="stats", bufs=4) as st_pool, \
             tc.tile_pool(name="small", bufs=6) as sm_pool, \
             tc.tile_pool(name="ppool", bufs=5) as p_pool, \
             tc.tile_pool(name="norm", bufs=4) as n_pool, \
             tc.tile_pool(name="sring", bufs=1, space="PSUM") as sring_ps, \
             tc.tile_pool(name="pvps", bufs=1, space="PSUM") as pv_ps:

            # qT/kT/vA are split into per-block / per-m-tile pool tiles:
            # one shared tensor makes Tile's dep tracker serialize readers
            # against the LATEST writer of the whole tensor (see spair_t).
            qTb = [persist.tile([128, 4, 512], BF16, name=f"qTb{i}")
                   for i in range(nblk)]
            kTt = [persist.tile([128, 4, 128], BF16, name=f"kTt{i}")
                   for i in range(nt_n)]
            vAt = [persist.tile([128, HG, HDIM + 1], BF16, name=f"vAt{i}")
                   for i in range(nt_n)]
            x1sb = persist.tile([128, KT, n_seq], BF16)
            x2sb = persist.tile([128, KT, n_seq], BF16)
            w_sb = {
                "q": persist.tile([128, KT, JW], BF16, name="wqsb"),
                "k": persist.tile([128, KT, JW], BF16, name="wksb"),
                "v": persist.tile([128, KT, JW], BF16, name="wvsb"),
            }
            for t in vAt:
                nc.vector.memset(t[:, :, HDIM:HDIM + 1], 1.0)
            zero_sb = persist.tile([128, 1], BF16, name="zero_sb")
            nc.vector.memset(zero_sb, 0.0)

            # Three 2-bank score-pair tiles. They are SEPARATE pool tiles
            # (not slices of one [128,6,512] ring): Tile's WAR tracking on
            # a shared tensor collapses to the latest reader, which made
            # every score pair wait on the previous m-tile's exp (measured
            # ring depth 1, sems $S>=N tracking exp(mi-1)). Separate
            # tensors give exactly pair-granular WAR: scores(mi) wait
            # exp(mi-3). Pair 2 doubles as the projection PSUM
            # double-buffer during block 0.
            spair_t = [sring_ps.tile([128, 2, 512], F32, name=f"spair{i}",
                                     tag=f"spair{i}") for i in range(3)]

            # ---- input DMAs. The startup critical path (wk + first x2/x1
            # columns) is split across four engine queues: one queue moves
            # ~90GB/s, so serializing the first 2MB there costs ~25us of
            # dead PE time at the start.
            xq = n_seq // 4
            wkr = wk_d.rearrange("(kt p) j -> p kt j", p=128)
            wqr = wq_d.rearrange("(kt p) j -> p kt j", p=128)
            nc.sync.dma_start(out=w_sb["k"][:, 0:4, :], in_=wkr[:, 0:4, :])
            nc.scalar.dma_start(out=w_sb["k"][:, 4:8, :], in_=wkr[:, 4:8, :])
            nc.gpsimd.dma_start(out=x2sb[:, :, 0:256], in_=x2r[:, :, 0:256])
            nc.gpsimd.dma_start(out=x2sb[:, :, 256:512], in_=x2r[:, :, 256:512])
            nc.sync.dma_start(out=w_sb["q"][:, 0:4, :], in_=wqr[:, 0:4, :])
            nc.scalar.dma_start(out=w_sb["q"][:, 4:8, :], in_=wqr[:, 4:8, :])
            # x1 rides scalar+gpsimd so the kT/qT transposes (sync queue)
            # don't sit behind a megabyte of x1
            nc.scalar.dma_start(out=x1sb[:, :, 0:256], in_=x1r[:, :, 0:256])
            nc.gpsimd.dma_start(out=x1sb[:, :, 256:512], in_=x1r[:, :, 256:512])
            for xi in range(1, 4):
                xs = slice(xi * xq, (xi + 1) * xq)
                nc.gpsimd.dma_start(out=x2sb[:, :, xs], in_=x2r[:, :, xs])
            nc.gpsimd.dma_start(
                out=w_sb["v"], in_=wv_d.rearrange("(kt p) j -> p kt j", p=128))
            for xi in range(1, 4):
                xs = slice(xi * xq, (xi + 1) * xq)
                nc.gpsimd.dma_start(out=x1sb[:, :, xs], in_=x1r[:, :, xs])

            proj_ctr = [0]
            # prologue projections fan out over all six PSUM slots (the
            # score ring is empty until the first attention step), so their
            # LN chains overlap instead of ping-ponging on two banks
            prolog_slots = [(2, 0), (0, 0), (1, 0), (2, 1), (0, 1), (1, 1)]

            def proj_matmuls(nm, xsb, nt):
                nsl = slice(nt * 128, (nt + 1) * 128)
                if prolog_slots:
                    pi, si = prolog_slots.pop(0)
                    ps = spair_t[pi][:, si, :]
                else:
                    ps = spair_t[2][:, proj_ctr[0] % 2, :]
                    proj_ctr[0] += 1
                for ct in range(KT):
                    nc.tensor.matmul(
                        ps, xsb[:, ct, nsl], w_sb[nm][:, ct, :],
                        start=(ct == 0), stop=(ct == KT - 1))
                return ps

            def stats_and_rstd(raw_hd, k_scaled, se):
                """Grouped stats + even/odd merge + bit-trick rsqrt.

                raw_hd is the bf16 SBUF drain of the projection tile (the
                PSUM bank is already free). Returns (rstd [128,8] f32,
                stats tile) where rstd is 1/sqrt(var+eps) for q
                (k_scaled=False) or 1/(8*sqrt(var+eps)) for k.
                """
                stats = st_pool.tile([128, HG, 6], F32, name="st", tag="st")
                # HW BNStats requires exactly 6 out elems/partition -> per head
                for hh in range(HG):
                    nc.vector.bn_stats(out=stats[:, hh, :], in_=raw_hd[:, hh, :])
                # The float scalar soup rides the otherwise-idle GPSIMD
                # (Q7 launch ~100ns/op); only the int bit-trick seed ops
                # stay on the DVE (Pool has no int ALU).
                me, mo = stats[:, :, 1], stats[:, :, 4]
                m2e, m2o = stats[:, :, 2], stats[:, :, 5]
                dlt = sm_pool.tile([128, HG], F32, name="dlt", tag="dlt")
                se.tensor_tensor(out=dlt, in0=me, in1=mo, op=ALU.subtract)
                ssum = sm_pool.tile([128, HG], F32, name="ssum", tag="ssum")
                se.tensor_tensor(out=ssum, in0=m2e, in1=m2o, op=ALU.add)
                d2 = sm_pool.tile([128, HG], F32, name="d2", tag="d2")
                se.tensor_tensor(out=d2, in0=dlt, in1=dlt, op=ALU.mult)
                # M2tot = 16*d2 + ssum = 64*var (Pool has no STT: two ops)
                d16 = sm_pool.tile([128, HG], F32, name="d16", tag="d16")
                se.tensor_scalar(
                    out=d16, in0=d2, scalar1=16.0, scalar2=None, op0=ALU.mult)
                m2t = sm_pool.tile([128, HG], F32, name="m2t", tag="m2t")
                se.tensor_tensor(out=m2t, in0=d16, in1=ssum, op=ALU.add)
                varx = sm_pool.tile([128, HG], F32, name="varx", tag="varx")
                if k_scaled:
                    # 64*(var+eps): rsqrt gives rstd/8 directly
                    se.tensor_scalar(
                        out=varx, in0=m2t, scalar1=64.0 * EPS, scalar2=None,
                        op0=ALU.add)
                else:
                    se.tensor_scalar(
                        out=varx, in0=m2t, scalar1=1.0 / 64.0, scalar2=EPS,
                        op0=ALU.mult, op1=ALU.add)
                # fast rsqrt: seed via int bit trick + 2 Newton iterations
                sh = sm_pool.tile([128, HG], I32, name="sh", tag="sh")
                nc.vector.tensor_scalar(
                    out=sh, in0=varx.bitcast(I32), scalar1=1, scalar2=None,
                    op0=ALU.logical_shift_right)
                shx = sm_pool.tile([128, HG], I32, name="shx", tag="shx")
                nc.vector.tensor_scalar(
                    out=shx, in0=sh, scalar1=-1, scalar2=None,
                    op0=ALU.bitwise_xor)
                seed = sm_pool.tile([128, HG], I32, name="seed", tag="seed")
                nc.vector.tensor_scalar(
                    out=seed, in0=shx, scalar1=0x5F3759E0, scalar2=None,
                    op0=ALU.add)
                y0 = seed.bitcast(F32)
                t1 = sm_pool.tile([128, HG], F32, name="t1", tag="t1")
                se.tensor_tensor(out=t1, in0=y0, in1=y0, op=ALU.mult)
                t2 = sm_pool.tile([128, HG], F32, name="t2", tag="t2")
                se.tensor_tensor(out=t2, in0=t1, in1=varx, op=ALU.mult)
                t3 = sm_pool.tile([128, HG], F32, name="t3", tag="t3")
                se.tensor_scalar(
                    out=t3, in0=t2, scalar1=-0.5, scalar2=1.5,
                    op0=ALU.mult, op1=ALU.add)
                y1 = sm_pool.tile([128, HG], F32, name="y1", tag="y1")
                se.tensor_tensor(out=y1, in0=y0, in1=t3, op=ALU.mult)
                # second Newton iteration (first alone leaves ~0.2% error,
                # which the exp amplifies to ~1.5e-2 on the output)
                u1 = sm_pool.tile([128, HG], F32, name="u1", tag="u1")
                se.tensor_tensor(out=u1, in0=y1, in1=y1, op=ALU.mult)
                u2 = sm_pool.tile([128, HG], F32, name="u2", tag="u2")
                se.tensor_tensor(out=u2, in0=u1, in1=varx, op=ALU.mult)
                u3 = sm_pool.tile([128, HG], F32, name="u3", tag="u3")
                se.tensor_scalar(
                    out=u3, in0=u2, scalar1=-0.5, scalar2=1.5,
                    op0=ALU.mult, op1=ALU.add)
                rstd = sm_pool.tile([128, HG], F32, name="rstd", tag="rstd")
                se.tensor_tensor(out=rstd, in0=y1, in1=u3, op=ALU.mult)
                return rstd, stats

            def xbars(stg, dst, nt):
                nc.sync.dma_start(out=dst, in_=stg, transpose=True)

            def kproj(nt, se=nc.gpsimd):
                ps = proj_matmuls("k", x2sb, nt)
                # single-op PSUM drain: the bank frees after this copy
                raw = stage.tile([128, JW], BF16, name="kraw", tag="kraw")
                nc.vector.tensor_copy(out=raw, in_=ps)
                raw_hd = raw.rearrange("p (h d) -> p h d", h=HG)
                rstd8, _ = stats_and_rstd(raw_hd, True, se)
                stg = stage.tile([128, JW], BF16, name="kstg", tag="kstg")
                stg_hd = stg.rearrange("p (h d) -> p h d", h=HG)
                se.tensor_tensor(
                    out=stg_hd, in0=raw_hd,
                    in1=rstd8.broadcast_to([128, HG, HDIM]), op=ALU.mult)
                xbars(stg, kTt[nt], nt)

            def qproj(nt, se=nc.gpsimd):
                ps = proj_matmuls("q", x1sb, nt)
                raw = stage.tile([128, JW], BF16, name="qraw", tag="qraw")
                nc.vector.tensor_copy(out=raw, in_=ps)
                raw_hd = raw.rearrange("p (h d) -> p h d", h=HG)
                rstd, stats = stats_and_rstd(raw_hd, False, se)
                # nmr = -mean*rstd; mean = (me+mo)/2
                msum = sm_pool.tile([128, HG], F32, name="msum", tag="msum")
                se.tensor_tensor(
                    out=msum, in0=stats[:, :, 1], in1=stats[:, :, 4], op=ALU.add)
                mh = sm_pool.tile([128, HG], F32, name="mh", tag="mh")
                se.tensor_scalar(
                    out=mh, in0=msum, scalar1=-0.5, scalar2=None, op0=ALU.mult)
                nmr = sm_pool.tile([128, HG], F32, name="nmr", tag="nmr")
                se.tensor_tensor(out=nmr, in0=mh, in1=rstd, op=ALU.mult)
                qtmp = stage.tile([128, JW], BF16, name="qtmp", tag="qtmp")
                qtmp_hd = qtmp.rearrange("p (h d) -> p h d", h=HG)
                se.tensor_tensor(
                    out=qtmp_hd, in0=raw_hd,
                    in1=rstd.broadcast_to([128, HG, HDIM]), op=ALU.mult)
                stg = stage.tile([128, JW], BF16, name="qstg", tag="qstg")
                stg_hd = stg.rearrange("p (h d) -> p h d", h=HG)
                se.tensor_tensor(
                    out=stg_hd, in0=qtmp_hd,
                    in1=nmr.broadcast_to([128, HG, HDIM]), op=ALU.add)
                xbars(stg, qTb[nt // 4][:, :, (nt % 4) * 128:
                                        (nt % 4 + 1) * 128], nt)

            def vproj(nt):
                ps = proj_matmuls("v", x2sb, nt)
                ps_hd = ps.rearrange("p (h d) -> p h d", h=HG)
                # gpsimd has no PSUM port; ACT has slack while projections
                # stream (block 0), so the drain cast rides ACT
                nc.scalar.copy(out=vAt[nt][:, :, 0:HDIM], in_=ps_hd)

            # ---------------- schedule ----------------
            # prologue: first k tiles + all q tiles for block 0. The soup
            # runs on the DVE here: nothing else competes yet and the
            # gpsimd per-op sem latency (~450ns effective) would put ~25us
            # of LN-chain latency in front of the first attention matmul.
            for nt in range(2):
                kproj(nt, se=nc.vector)
            for nt in range(4):
                qproj(nt, se=nc.vector)

            # filler projection work slotted into (b, hp, mi) gaps:
            # k and v tiles just-in-time under (b0, hp0); the remaining
            # q tiles spread over (b0, hp1..3), done before block 1
            fillers = {}
            for mi in range(nt_n):
                if mi + 2 < nt_n:
                    fillers.setdefault((0, 0, mi), []).append(("k", mi + 2))
                fillers.setdefault((0, 0, mi), []).append(("v", mi))
            for hp in range(1, 4):
                for i in range(4):
                    nt = 4 * hp + i
                    if nt < nt_n:
                        fillers.setdefault((0, hp, 3 * i + 1), []).append(
                            ("q", nt))

            # Flat global stream over (b, hp, mi): the PV/drain trail runs
            # LAG steps behind the score/exp head and crosses (hp, b)
            # boundaries, so block drains interleave with the next block's
            # scores and the PE queue never breaks (a broken queue means a
            # HAM re-throttle to 1.2 GHz). LAG=5 also gives the boundary
            # drain copies time to free the pv banks before the next
            # block's start=True PV matmul needs them.
            LAG = 5
            steps = [(b, hp, mi) for b in range(nblk)
                     for hp in range(HG // 2) for mi in range(nt_n)]
            pv_state = {}
            p_tiles = {}
            gp = [0]

            def emit_pv(gi):
                b2, hp2, m2 = steps[gi]
                he, ho = 2 * hp2, 2 * hp2 + 1
                if m2 == 0:
                    pv_state[(b2, hp2)] = [
                        pv_ps.tile([65, 512], F32, name=f"pv{i}",
                                   tag=f"pv{i}") for i in range(2)]
                pvs = pv_state[(b2, hp2)]
                p = p_tiles.pop(gi)
                nc.tensor.matmul(pvs[0], vAt[m2][:, he, :], p[:, 0, :],
                                 start=(m2 == 0), stop=(m2 == nt_n - 1))
                nc.tensor.matmul(pvs[1], vAt[m2][:, ho, :], p[:, 1, :],
                                 start=(m2 == 0), stop=(m2 == nt_n - 1))
                if m2 == nt_n - 1:
                    # drain numerators + denominator rows (split ACT/DVE);
                    # host does the final divide
                    nsl2 = slice(b2 * 512, (b2 + 1) * 512)
                    for i, h in enumerate((he, ho)):
                        osb = n_pool.tile([65, 512], F32, name=f"osb{i}",
                                          tag=f"osb{i}", bufs=2)
                        if i == 0:
                            nc.scalar.copy(out=osb, in_=pvs[i])
                        else:
                            nc.vector.tensor_copy(out=osb, in_=pvs[i])
                        nc.gpsimd.dma_start(out=out_d[h, :, nsl2], in_=osb)
                    del pv_state[(b2, hp2)]

            def ballast(gi, n_free):
                # The warm (2.4 GHz) steady state has near-zero slack: exp
                # capacity per m-tile ~= PE work per m-tile, so any jitter
                # stalls the PE and HAM re-throttles the array to 1.2 GHz
                # (measured: the cold state is the stable attractor). A tiny
                # M=1 accumulate-zero matmul into the spare partition-64 row
                # span of the live pv bank keeps the PE array busy through
                # dependency jitter; it adds exactly +0.0 to denominator
                # columns that the start=True matmul overwrites anyway.
                ti = gi - LAG
                if ti < 0:
                    return
                b2, hp2, m2 = steps[ti]
                if m2 >= nt_n - 2 or (b2, hp2) not in pv_state:
                    return
                dmy = pv_state[(b2, hp2)][gi % 2]
                nc.tensor.matmul(
                    dmy[64:65, 0:n_free], zero_sb, x2sb[:, 0, 0:n_free],
                    start=False, stop=False, skip_group_check=True)

            for gi, (b, hp, mi) in enumerate(steps):
                nsl = slice(b * 512, (b + 1) * 512)
                # block 0: slots 4,5 are the projection double-buffer ->
                # rotate over 2 pairs; afterwards over all 3
                npairs = 2 if b == 0 else 3
                dve_share = DVE_EXP_SHARE[min(b, 3)]
                pair = spair_t[gp[0] % npairs]
                gp[0] += 1
                msl = slice(mi * 128, (mi + 1) * 128)
                # row-tiled pair: tile_position (0,0)/(64,0) auto-derived
                # from base partitions -> the PE runs both heads' scores
                # concurrently
                nc.tensor.matmul(
                    pair[:, 0, :], kTt[mi][0:64, hp, :],
                    qTb[b][0:64, hp, :], start=True, stop=True)
                nc.tensor.matmul(
                    pair[:, 1, :], kTt[mi][64:128, hp, :],
                    qTb[b][64:128, hp, :], start=True, stop=True)
                spair = pair[:, :, :]
                p = p_pool.tile([128, 2, 512], BF16, name="p",
                                tag="p", bufs=LAG + 3)
                use_dve = ((mi * dve_share) // nt_n
                           != ((mi + 1) * dve_share) // nt_n)
                if use_dve:
                    # Schraudolph: bf16 bits of ~exp(s) via one f32
                    # round-to-int16 affine (err <= 3%)
                    nc.vector.tensor_scalar(
                        out=p.bitcast(I16), in0=spair,
                        scalar1=EXP_S1, scalar2=EXP_S2,
                        op0=ALU.mult, op1=ALU.add)
                else:
                    nc.scalar.activation(
                        out=p, in_=spair,
                        func=mybir.ActivationFunctionType.Exp,
                        scale=1.0)
                p_tiles[gi] = p
                for kind, nt in fillers.get((b, hp, mi), ()):
                    if kind == "v":
                        vproj(nt)
                    elif kind == "k":
                        kproj(nt)
                    else:
                        qproj(nt)
                if gi >= LAG:
                    emit_pv(gi - LAG)
            for gi in range(len(steps) - LAG, len(steps)):
                emit_pv(gi)

    split_multi_waits(nc)
    return nc


def build_general(n_seq=NSEQ, has_bq=False, has_bkv=False, has_gbq=False, has_gbk=False):
    nt_n = n_seq // 128        # n tiles (16)
    sw = min(1024, n_seq)      # s-tile width (ACT exp granularity)
    nblk = n_seq // sw         # n blocks per head
    nch = sw // 512            # 512-wide output chunks per block
    scale = 1.0 / np.sqrt(HDIM)

    nc = bass.Bass()
    x1t = nc.dram_tensor("x1t", [DIM, n_seq], BF16, kind="ExternalInput")
    x2t = nc.dram_tensor("x2t", [DIM, n_seq], BF16, kind="ExternalInput")
    wq_d = nc.dram_tensor("wq", [DIM, JW], BF16, kind="ExternalInput")
    wk_d = nc.dram_tensor("wk", [DIM, JW], BF16, kind="ExternalInput")
    wv_d = nc.dram_tensor("wv", [DIM, JW], BF16, kind="ExternalInput")
    eye_d = nc.dram_tensor("eye", [128, 128], BF16, kind="ExternalInput")
    if has_bq:
        bq_d = nc.dram_tensor("bq", [JW], F32, kind="ExternalInput")
    if has_bkv:
        bk_d = nc.dram_tensor("bk", [JW], F32, kind="ExternalInput")
        bv_d = nc.dram_tensor("bv", [JW], F32, kind="ExternalInput")
    if has_gbq:
        gq_d = nc.dram_tensor("gq", [JW], F32, kind="ExternalInput")
        betq_d = nc.dram_tensor("betq", [JW], F32, kind="ExternalInput")
    if has_gbk:
        gk_d = nc.dram_tensor("gk", [JW], F32, kind="ExternalInput")
        betk_d = nc.dram_tensor("betk", [JW], F32, kind="ExternalInput")
    out_d = nc.dram_tensor("outT", [JW, n_seq], F32, kind="ExternalOutput")

    def bcast_from_dram(pool, vec_d, name):
        t = pool.tile([128, JW], F32, name=name)
        src = bass.AP(tensor=vec_d.tensor, offset=vec_d.offset,
                      ap=[[0, 128]] + list(vec_d.ap))
        nc.sync.dma_start(out=t, in_=src)
        return t

    with tile.TileContext(nc) as tc:
        with tc.tile_pool(name="persist", bufs=1) as persist:
            qT = persist.tile([128, 4, n_seq], BF16)   # [j, n] post-LN q
            kT = persist.tile([128, 4, n_seq], BF16)
            vA = persist.tile([128, nt_n, HG, HDIM + 1], BF16)  # v + ones col
            eye_sb = persist.tile([128, 128], BF16)
            eps_sb = persist.tile([128, 1], F32)
            nc.sync.dma_start(out=eye_sb, in_=eye_d[:, :])
            nc.vector.memset(eps_sb, EPS)
            nc.vector.memset(vA[:, :, :, HDIM:HDIM + 1], 1.0)

            bqb = bcast_from_dram(persist, bq_d[:], "bqb") if has_bq else None
            bkb = bcast_from_dram(persist, bk_d[:], "bkb") if has_bkv else None
            bvb = bcast_from_dram(persist, bv_d[:], "bvb") if has_bkv else None
            gqb = bcast_from_dram(persist, gq_d[:], "gqb") if has_gbq else None
            btqb = bcast_from_dram(persist, betq_d[:], "btqb") if has_gbq else None
            gkb = bcast_from_dram(persist, gk_d[:], "gkb") if has_gbk else None
            btkb = bcast_from_dram(persist, betk_d[:], "btkb") if has_gbk else None

            # ---------------- projection + LN + transpose ----------------
            with tc.tile_pool(name="wpool", bufs=1) as wpool, \
                 tc.tile_pool(name="lnb", bufs=6) as ln_pool, \
                 tc.tile_pool(name="stats", bufs=6) as st_pool, \
                 tc.tile_pool(name="pps", bufs=6, space="PSUM") as proj_ps, \
                 tc.tile_pool(name="tps", bufs=2, space="PSUM") as tp_ps:

                w_sb = {}
                for nm, dram in (("q", wq_d), ("k", wk_d), ("v", wv_d)):
                    w_sb[nm] = wpool.tile([128, KT, JW], BF16, name=f"w_{nm}")
                x1sb = wpool.tile([128, KT, n_seq], BF16, name="x1sb")
                x2sb = wpool.tile([128, KT, n_seq], BF16, name="x2sb")
                # DMA order matters: the first q matmul chain needs w_q and
                # the first x1 chunk only, so those go first
                xq = n_seq // 4
                x1r = x1t.rearrange("(kt p) n -> p kt n", p=128)
                x2r = x2t.rearrange("(kt p) n -> p kt n", p=128)
                # the first q matmul chain needs only w_q and x1 cols 0:128;
                # land those first so the PE starts ~5us in
                nc.sync.dma_start(
                    out=w_sb["q"],
                    in_=wq_d.rearrange("(kt p) j -> p kt j", p=128))
                nc.sync.dma_start(out=x1sb[:, :, 0:128], in_=x1r[:, :, 0:128])
                nc.sync.dma_start(
                    out=w_sb["k"],
                    in_=wk_d.rearrange("(kt p) j -> p kt j", p=128))
                nc.sync.dma_start(out=x2sb[:, :, 0:128], in_=x2r[:, :, 0:128])
                nc.sync.dma_start(
                    out=w_sb["v"],
                    in_=wv_d.rearrange("(kt p) j -> p kt j", p=128))
                if xq > 128:
                    nc.sync.dma_start(out=x1sb[:, :, 128:xq],
                                      in_=x1r[:, :, 128:xq])
                    nc.sync.dma_start(out=x2sb[:, :, 128:xq],
                                      in_=x2r[:, :, 128:xq])
                for xi in range(1, 4):
                    xs = slice(xi * xq, (xi + 1) * xq)
                    nc.sync.dma_start(out=x1sb[:, :, xs], in_=x1r[:, :, xs])
                    nc.sync.dma_start(out=x2sb[:, :, xs], in_=x2r[:, :, xs])

                def layernorm_into(psum, dst, bias_b, gb, bb_, use_act):
                    # per-head LN of a [128, 512] projection tile
                    if bias_b is not None:
                        src = ln_pool.tile([128, JW], F32, name="biased",
                                           tag="biased")
                        nc.vector.tensor_add(out=src, in0=psum, in1=bias_b)
                    else:
                        src = psum
                    stats = st_pool.tile([128, HG, 6], F32, name="stats")
                    for h in range(HG):
                        nc.vector.bn_stats(
                            out=stats[:, h, :],
                            in_=src[:, h * HDIM:(h + 1) * HDIM])
                    mv = st_pool.tile([128, HG, 2], F32, name="mv")
                    for h in range(HG):
                        nc.vector.bn_aggr(out=mv[:, h, :], in_=stats[:, h, :])
                    std = st_pool.tile([128, HG], F32, name="std")
                    nc.scalar.activation(
                        out=std, in_=mv[:, :, 1],
                        func=mybir.ActivationFunctionType.Sqrt,
                        bias=eps_sb, scale=1.0)
                    rstd = st_pool.tile([128, HG], F32, name="rstd")
                    nc.vector.reciprocal(out=rstd, in_=std)
                    # bias for the ACT apply: -mean * rstd
                    negmr = st_pool.tile([128, HG], F32, name="negmr")
                    nc.vector.tensor_mul(out=negmr, in0=mv[:, :, 0],
                                         in1=rstd)
                    nc.vector.tensor_scalar(
                        out=negmr, in0=negmr, scalar1=-1.0, scalar2=None,
                        op0=mybir.AluOpType.mult)
                    for h in range(HG):
                        # (q-mu)*rstd == q*rstd + (-mu*rstd), one ACT op
                        nc.scalar.activation(
                            out=dst[:, h * HDIM:(h + 1) * HDIM],
                            in_=src[:, h * HDIM:(h + 1) * HDIM],
                            func=mybir.ActivationFunctionType.Identity,
                            bias=negmr[:, h:h + 1], scale=rstd[:, h:h + 1])
                    if gb is not None:
                        nc.vector.tensor_mul(out=dst, in0=dst, in1=gb)
                        nc.vector.tensor_add(out=dst, in0=dst, in1=bb_)

                def emit_transposes(ln, dstT, nt):
                    nsl = slice(nt * 128, (nt + 1) * 128)
                    for jt in range(4):
                        tp = tp_ps.tile([128, 128], BF16, name="tp", tag="tp")
                        nc.tensor.transpose(
                            tp, ln[:, jt * 128:(jt + 1) * 128], eye_sb)
                        # split explicitly: nc.any routes all of these to the
                        # already-saturated ACT (the projection pacer)
                        if jt % 2 == 0:
                            nc.vector.tensor_copy(out=dstT[:, jt, nsl], in_=tp)
                        else:
                            nc.scalar.copy(out=dstT[:, jt, nsl], in_=tp)

                # transposes run one n-tile behind the matmuls so the PE
                # never waits on a just-computed LN result
                pending = []
                for nt in range(nt_n):
                    nsl = slice(nt * 128, (nt + 1) * 128)
                    x1c = x1sb[:, :, nsl]
                    x2c = x2sb[:, :, nsl]

                    for nm, xc, dstT, bias_b, gb, bb_ in (
                        ("q", x1c, qT, bqb, gqb, btqb),
                        ("k", x2c, kT, bkb, gkb, btkb),
                    ):
                        ps = proj_ps.tile([128, JW], F32, name="ps", tag="ps")
                        for ct in range(KT):
                            nc.tensor.matmul(
                                ps, xc[:, ct, :], w_sb[nm][:, ct, :],
                                start=(ct == 0), stop=(ct == KT - 1))
                        ln = ln_pool.tile([128, JW], BF16, name="ln", tag="ln")
                        layernorm_into(ps, ln, bias_b, gb, bb_, True)
                        pending.append((ln, dstT, nt))

                    ps = proj_ps.tile([128, JW], F32, name="ps", tag="ps")
                    for ct in range(KT):
                        nc.tensor.matmul(
                            ps, x2c[:, ct, :], w_sb["v"][:, ct, :],
                            start=(ct == 0), stop=(ct == KT - 1))
                    psg = ps.rearrange("p (h d) -> p h d", h=HG)
                    if bvb is not None:
                        nc.vector.tensor_add(
                            out=vA[:, nt, :, 0:HDIM], in0=psg,
                            in1=bvb.rearrange("p (h d) -> p h d", h=HG))
                    else:
                        nc.vector.tensor_copy(out=vA[:, nt, :, 0:HDIM], in_=psg)
                    while len(pending) > 2:
                        emit_transposes(*pending.pop(0))
                for args in pending:
                    emit_transposes(*args)

            # ---------------- attention ----------------
            with tc.tile_pool(name="sps", bufs=2, space="PSUM") as s_ps, \
                 tc.tile_pool(name="pvps", bufs=2, space="PSUM") as pv_ps, \
                 tc.tile_pool(name="psb", bufs=3) as p_pool, \
                 tc.tile_pool(name="nrm", bufs=3) as n_pool, \
                 tc.tile_pool(name="dsc", bufs=4, space="DRAM") as dram_pool:
                for h in range(HG):
                    pt, bp = divmod(h, 2)
                    prows = slice(bp * 64, (bp + 1) * 64)
                    kTh = kT[prows, pt, :]
                    qTh = qT[prows, pt, :]
                    for blk in range(nblk):
                        pvs = [pv_ps.tile([65, 512], F32, name=f"pv{c2}",
                                          tag="pv") for c2 in range(nch)]
                        # software pipeline: PV matmuls run two m-tiles behind
                        # the score matmuls, so the PE queue never stalls on
                        # ACT's exp (a stalled PE FIFO keeps HAM at 1.2 GHz)
                        LAG = 2
                        p_tiles = {}
                        for mi in range(nt_n + LAG):
                            if mi < nt_n:
                                s = s_ps.tile([128, sw], F32, name="s",
                                              tag="s", bufs=LAG + 1)
                                for c2 in range(nch):
                                    n0 = blk * sw + c2 * 512
                                    nc.tensor.matmul(
                                        s[:, c2 * 512:(c2 + 1) * 512],
                                        kTh[:, mi * 128:(mi + 1) * 128],
                                        qTh[:, n0:n0 + 512],
                                        start=True, stop=True)
                                p = p_pool.tile([128, sw], BF16, name="p",
                                                tag="p", bufs=LAG + 2)
                                nc.scalar.activation(
                                    out=p, in_=s,
                                    func=mybir.ActivationFunctionType.Exp,
                                    scale=float(scale))
                                p_tiles[mi] = p
                            if mi >= LAG:
                                m = mi - LAG
                                p = p_tiles.pop(m)
                                for c2 in range(nch):
                                    nc.tensor.matmul(
                                        pvs[c2], vA[:, m, h, :],
                                        p[:, c2 * 512:(c2 + 1) * 512],
                                        start=(m == 0), stop=(m == nt_n - 1))
                        # Drain numerators + denominator rows out of PSUM
                        # first (frees the pv banks for the next block), then
                        # one batched reciprocal per block: rows parked at
                        # partitions 0/32 (compute APs need 32-aligned bases).
                        # The partition-broadcast goes through a DRAM bounce
                        # (SBUF APs cannot have a zero partition step) so the
                        # PE never has to wait on this chain.
                        dens = n_pool.tile([64, 512], F32, name="dens",
                                           tag="dens")
                        nc.vector.memset(dens, 1.0)
                        osb0s = []
                        for c2 in range(nch):
                            osb0 = n_pool.tile([64, 512], F32, name="osb0",
                                               tag="osb0", bufs=4)
                            nc.vector.tensor_copy(out=osb0, in_=pvs[c2][0:64, :])
                            nc.scalar.copy(out=dens[32 * c2:32 * c2 + 1, :],
                                           in_=pvs[c2][64:65, :])
                            osb0s.append(osb0)
                        denr = n_pool.tile([64, 512], F32, name="denr",
                                           tag="denr")
                        nc.vector.reciprocal(out=denr, in_=dens)
                        for c2 in range(nch):
                            dscr = dram_pool.tile([512], F32, name="dscr",
                                                  tag="dscr")
                            nc.sync.dma_start(
                                out=dscr, in_=denr[32 * c2:32 * c2 + 1, :])
                            denb_sb = n_pool.tile([64, 512], F32,
                                                  name="denb_sb",
                                                  tag="denb_sb")
                            dscr_b = bass.AP(tensor=dscr.tensor,
                                             offset=dscr.offset,
                                             ap=[[0, 64]] + list(dscr.ap))
                            nc.sync.dma_start(out=denb_sb, in_=dscr_b)
                            osb = n_pool.tile([64, 512], F32, name="osb",
                                              tag="osb")
                            nc.vector.tensor_mul(
                                out=osb, in0=osb0s[c2], in1=denb_sb)
                            n0 = blk * sw + c2 * 512
                            nc.sync.dma_start(
                                out=out_d[h * HDIM:(h + 1) * HDIM,
                                          n0:n0 + 512],
                                in_=osb)
    split_multi_waits(nc)
    return nc



def shard_inputs_general(x1, x2, wq, bq, wkv, bkv, gamma_q, beta_q, gamma_k, beta_k,
                 flags, n_seq=NSEQ):
    has_bq, has_bkv, has_gbq, has_gbk = flags
    bf16 = ml_dtypes.bfloat16
    eye = np.eye(128, dtype=bf16)
    in_maps = []
    for core in range(NCORES):
        b, g = divmod(core, 2)
        jsl = slice(g * JW, (g + 1) * JW)
        m = {
            "x1t": np.ascontiguousarray(x1[b, :n_seq].T.astype(bf16)),
            "x2t": np.ascontiguousarray(x2[b, :n_seq].T.astype(bf16)),
            "wq": np.ascontiguousarray(wq[:, jsl].astype(bf16)),
            "wk": np.ascontiguousarray(wkv[:, jsl].astype(bf16)),
            "wv": np.ascontiguousarray(
                wkv[:, DIM + g * JW:DIM + (g + 1) * JW].astype(bf16)),
            "eye": eye,
        }
        if has_bq:
            m["bq"] = np.ascontiguousarray(bq[jsl])
        if has_bkv:
            m["bk"] = np.ascontiguousarray(bkv[jsl])
            m["bv"] = np.ascontiguousarray(bkv[DIM + g * JW:DIM + (g + 1) * JW])
        if has_gbq:
            m["gq"] = np.tile(gamma_q, HG).astype(np.float32)
            m["betq"] = np.tile(beta_q, HG).astype(np.float32)
        if has_gbk:
            m["gk"] = np.tile(gamma_k, HG).astype(np.float32)
            m["betk"] = np.tile(beta_k, HG).astype(np.float32)
        in_maps.append(m)
    return in_maps



def shard_inputs_fast(x1, x2, wq, wkv, n_seq=NSEQ):
    bf16 = ml_dtypes.bfloat16
    in_maps = []
    for core in range(NCORES):
        b, g = divmod(core, 2)
        jsl = slice(g * JW, (g + 1) * JW)
        m = {
            "x1t": np.ascontiguousarray(x1[b, :n_seq].T.astype(bf16)),
            "x2t": np.ascontiguousarray(x2[b, :n_seq].T.astype(bf16)),
            "wq": np.ascontiguousarray(wq[:, jsl].astype(bf16)),
            "wk": np.ascontiguousarray(wkv[:, jsl].astype(bf16)),
            "wv": np.ascontiguousarray(
                wkv[:, DIM + g * JW:DIM + (g + 1) * JW].astype(bf16)),
        }
        in_maps.append(m)
    return in_maps


def kernel(x1, x2, wq, bq, wkv, bkv, gamma_q, beta_q, gamma_k, beta_k):
    global LAST_RESULTS
    x1 = np.asarray(x1, dtype=np.float32)
    x2 = np.asarray(x2, dtype=np.float32)
    wq = np.asarray(wq, dtype=np.float32)
    bq = np.asarray(bq, dtype=np.float32)
    wkv = np.asarray(wkv, dtype=np.float32)
    bkv = np.asarray(bkv, dtype=np.float32)
    gamma_q = np.asarray(gamma_q, dtype=np.float32)
    beta_q = np.asarray(beta_q, dtype=np.float32)
    gamma_k = np.asarray(gamma_k, dtype=np.float32)
    beta_k = np.asarray(beta_k, dtype=np.float32)

    nondefault = (bool(np.any(bq)) or bool(np.any(bkv))
                  or not (np.all(gamma_q == 1.0) and np.all(beta_q == 0.0))
                  or not (np.all(gamma_k == 1.0) and np.all(beta_k == 0.0)))
    if nondefault:
        # general path (biases / non-default gamma,beta): v1 kernel
        flags = (
            bool(np.any(bq)),
            bool(np.any(bkv)),
            not (np.all(gamma_q == 1.0) and np.all(beta_q == 0.0)),
            not (np.all(gamma_k == 1.0) and np.all(beta_k == 0.0)),
        )
        nc = build_general(NSEQ, *flags)
        in_maps = shard_inputs_general(x1, x2, wq, bq, wkv, bkv, gamma_q,
                                       beta_q, gamma_k, beta_k, flags)
        trace = bool(int(os.environ.get("KERNEL_TRACE", "0")))
        res = run_bass_kernel_spmd(nc, in_maps, core_ids=list(range(NCORES)),
                                   trace=trace)
        LAST_RESULTS = res
        out = np.empty((B, NSEQ, DIM), dtype=np.float32)
        for core in range(NCORES):
            b, g = divmod(core, 2)
            out[b, :, g * JW:(g + 1) * JW] = res.results[core]["outT"].T
        return out

    nc = build_fast(NSEQ)
    in_maps = shard_inputs_fast(x1, x2, wq, wkv)
    trace = bool(int(os.environ.get("KERNEL_TRACE", "0")))
    res = run_bass_kernel_spmd(nc, in_maps, core_ids=list(range(NCORES)),
                               trace=trace)
    LAST_RESULTS = res
    out = np.empty((B, NSEQ, DIM), dtype=np.float32)
    for core in range(NCORES):
        b, g = divmod(core, 2)
        r = res.results[core]["out2"]          # [HG, 65, NSEQ]
        o = r[:, 0:HDIM, :] / r[:, HDIM:HDIM + 1, :]
        out[b, :, g * JW:(g + 1) * JW] = (
            o.transpose(2, 0, 1).reshape(NSEQ, JW))
    return out

